# revision 49
# baseline (speedup 1.0000x reference)
"""Trainium2 Bass kernel for the DAGH sample loss.

loss = 0.5 * tr_loss / n^2 * 1e4 + 0.5 * bla_loss / n + 0.5 * oth_loss / K

with
  tr_loss  = dot(rowsum(w), fn) + dot(colsum(w), bn) - 2 * sum((F @ w) * B)
  oth_loss = ||F F^T / n - I||_F^2
  bla_loss = sum_k (sum_i F[k, i])^2

Strategy (8 cores, SPMD): the problem is memory-bound on streaming w
(256 MB in f32), so w is quantized host-side to fp8-e4m3 (the 2e-2
rel-err gate leaves ~3 orders of magnitude of headroom; measured impact
is ~3e-5) and sharded COLUMN-wise: core c owns w[:, c*1024:(c+1)*1024],
pre-transposed on host so each SBUF partition receives contiguous 8 KB
DMA lines (32 KB partition stride measured fastest; the whole 8.4 MB
shard is SBUF-resident).

All three w-dependent reductions collapse into one augmented fp8
matmul chain per core, running in DoubleRow mode (two 128-row k-tiles
per instruction, 0.5 PE cycles/row):

  Faug = [F; ones; fn - 64]  (66 x 8192, fp8)  ->  out = Faug @ w_cols

  rows 0..63 . B          -> cross partial
  row  64    . (bn + 64)  -> colsum-dot partial + 64 * sum(w)
  row  65    . ones       -> rowsum-dot partial - 64 * sum(w)

(fn is centered by 64 so fp8 keeps ~6x more precision on it; the
64*sum(w) terms cancel exactly in tr_loss = S[64] + S[65] - 2*cross.)
The contraction is split into two PSUM groups so the first group's
multiply-by-Baug + reduce runs mid-stream, hidden under DMA; the final
chunks taper (8,8,8,4,2,2 blocks) so almost no matmul work trails the
last DMA byte.  Gram = F F^T and the F row-sums come from one extra
fp8 matmul chain over the same resident Faug^T tiles (every core
computes the identical full Gram; the host uses core 0's).  Host
combines the per-core scalar partials in f64.
"""

import numpy as np

BATCH = 8192
K = 64
NCORES = 8
COLS = BATCH // NCORES  # w columns per core (column sharding)
KP = 128  # partition dim
NB = BATCH // KP  # 64 row-blocks of w = full contraction per core
HB = NB // 2  # blocks per half-tensor (32 KB partition stride)
JT = 512  # psum free-dim tile
NJ = COLS // JT  # psum tiles per core (2)
M = K + 2  # augmented lhs rows (F, ones, fn-64)
MP = 80  # ft block stride: dual-fp8 ldweights needs k-subtile step % 16 == 0
CB = 8  # row-blocks per w DMA chunk (8 KB/partition lines)
NG = 2  # psum accumulation groups (= the two half-tensors)

_compiled = {}


def _chunk_sched(cb, taper):
    """Per-half chunk sizes in blocks.  half 1 optionally tapers so the
    final DMA chunks are small and almost no compute trails them."""
    uniform = [cb] * (HB // cb)
    if not taper:
        return [uniform, list(uniform)]
    tail = [cb // 2, cb // 4, cb // 4] if cb >= 4 else [cb]
    head = [cb] * ((HB - sum(tail)) // cb)
    assert sum(head) + sum(tail) == HB
    return [uniform, head + tail]


def _build(
    loop_reps=1,
    runtime_reps=False,
    dma_only=False,
    dma_tile=None,
    loop_whole=False,
    cb=CB,
    no_dve=False,
    no_tail=False,
    taper=True,
    dve_mode="mr16",
    unroll=1,
    gram_in_loop=False,
):
    """loop_reps > 1 (or runtime_reps=True with a "reps" input tensor)
    wraps the main stream in a hardware For_i loop that recomputes
    identical results -- used only by test.py to time the steady-state
    stream without per-call NEFF-load overhead.  dma_only strips compute
    to measure the pure DMA bandwidth ceiling; no_dve / no_tail strip
    the tail stages for profiling."""
    import contextlib

    import concourse.bacc as bacc
    import concourse.mybir as mybir
    import concourse.tile as tile

    f32 = mybir.dt.float32
    bf16 = mybir.dt.bfloat16
    fp8 = mybir.dt.float8e4
    DR = mybir.MatmulPerfMode.DoubleRow

    sched = _chunk_sched(cb, taper)

    nc = bacc.Bacc(
        "TRN2", target_bir_lowering=False, debug=False, num_devices=NCORES
    )

    # w shard pre-transposed host-side, split into two half-tensors so the
    # DMA partition stride is 32 KB (measured faster than 64 KB):
    # wh[h, p, b*COLS + j] = w_cols[(h*HB + b)*128 + p, j]
    wh_d = nc.dram_tensor(
        "wh", [2, KP, HB * COLS], fp8, kind="ExternalInput"
    ).ap()
    # full Faug^T tiles: ft[p, b, m] = Faug[m, b*128 + p]  (m padded to MP)
    ft_d = nc.dram_tensor("ft", [KP, NB, MP], fp8, kind="ExternalInput").ap()
    # Baug column slice: [B; bn + 64; ones] for this core's columns
    baug_d = nc.dram_tensor(
        "baug", [M, COLS], bf16, kind="ExternalInput"
    ).ap()
    reps_d = None
    if runtime_reps:
        reps_d = nc.dram_tensor(
            "reps", [1, 2], mybir.dt.int32, kind="ExternalInput"
        ).ap()
    acc_d = nc.dram_tensor(
        "acc", [M, NG * NJ], f32, kind="ExternalOutput"
    ).ap()
    gram_d = nc.dram_tensor("gram", [K, M], f32, kind="ExternalOutput").ap()

    with tile.TileContext(nc) as tc:
        with (
            tc.tile_pool(name="persist", bufs=1) as persist,
            tc.tile_pool(name="wp", bufs=unroll) as wp,
            tc.tile_pool(name="scratch", bufs=2) as scratch,
            tc.tile_pool(name="psum", bufs=NG * NJ, space="PSUM") as psum,
            tc.tile_pool(name="psum_small", bufs=1, space="PSUM") as psum_small,
        ):
            ft_sb = persist.tile([KP, NB, MP], fp8, name="ft_sb")
            baug_sb = persist.tile([M, COLS], bf16, name="baug_sb")
            acc_sb = persist.tile([M, NG * NJ], f32, name="acc_sb")
            if dma_only or no_dve or no_tail:
                nc.vector.memset(acc_sb, 0.0)

            # preamble on the scalar-engine HWDGE ring: keeps these loads
            # off the sync ring so the w chunks aren't queued behind them
            nc.scalar.dma_start(out=ft_sb, in_=ft_d)
            nc.scalar.dma_start(out=baug_sb, in_=baug_d)

            # gram + F row-sums, fused: out[k, m] = sum_i F[k,i]*Faug[m,i]
            # (cols 0..63 = F F^T, col 64 = rowsum(F), col 65 ignored).
            # Tensor-engine work that overlaps the first w chunk DMAs.
            # Normal mode: DoubleRow loses below FD=128 (ldweights dominates).
            def gram_chain():
                gram_pt = psum_small.tile([K, M], f32, name="gram_pt")
                for b in range(NB):
                    nc.tensor.matmul(
                        gram_pt,
                        lhsT=ft_sb[:, b, 0:K],
                        rhs=ft_sb[:, b, 0:M],
                        start=(b == 0),
                        stop=(b == NB - 1),
                    )
                gram_sb = persist.tile([K, M], f32, name="gram_sb")
                nc.vector.tensor_copy(gram_sb, gram_pt)
                nc.scalar.dma_start(out=gram_d, in_=gram_sb)

            if not gram_in_loop:
                gram_chain()

            def dve_tail(g, pts):
                # multiply group g's completed psum by Baug and reduce over
                # the free dim into acc.  (baseline found fused
                # tensor_tensor_reduce faults on HW with a PSUM input, so
                # the default "mr" keeps multiply and reduce separate)
                for s in range(NJ):
                    bslice = baug_sb[:, s * JT : (s + 1) * JT]
                    acol = acc_sb[:, g * NJ + s : g * NJ + s + 1]
                    if dve_mode == "actmr":
                        # ACT engine drains PSUM -> SBUF as bf16 (2x DVE
                        # rate downstream); DVE multiplies and reduces.
                        # The two engines pipeline across the two s tiles.
                        st = scratch.tile([M, JT], bf16, name="act_out")
                        nc.scalar.copy(st, pts[g][s])
                        st2 = scratch.tile([M, JT], bf16, name="mul_out")
                        nc.vector.tensor_mul(st2, st, bslice)
                        nc.vector.tensor_reduce(
                            out=acol,
                            in_=st2,
                            axis=mybir.AxisListType.X,
                            op=mybir.AluOpType.add,
                        )
                    elif dve_mode == "mr16":
                        # like mr but the product scratch is bf16, halving
                        # the DVE reduce pass
                        st = scratch.tile([M, JT], bf16, name="mul_out")
                        nc.vector.tensor_mul(st, pts[g][s], bslice)
                        nc.vector.tensor_reduce(
                            out=acol,
                            in_=st,
                            axis=mybir.AxisListType.X,
                            op=mybir.AluOpType.add,
                        )
                    else:
                        # "mr": separate f32 multiply and reduce.  (The
                        # fused tensor_tensor_reduce instruction hangs this
                        # hardware even with SBUF-only inputs — do not use.)
                        st = scratch.tile([M, JT], f32, name="mul_out32")
                        nc.vector.tensor_mul(st, pts[g][s], bslice)
                        nc.vector.tensor_reduce(
                            out=acol,
                            in_=st,
                            axis=mybir.AxisListType.X,
                            op=mybir.AluOpType.add,
                        )

            def stream():
                wl = wp.tile([KP, NB, COLS], fp8, name="wl")
                pts = [
                    [psum.tile([M, JT], f32, name="mm_out") for _ in range(NJ)]
                    for _ in range(NG)
                ]
                for h in range(2):
                    off = 0  # block offset within this half
                    a_lo, a_hi = h * HB // 2, (h + 1) * HB // 2
                    for ci, nb in enumerate(sched[h]):
                        b0 = h * HB + off  # absolute block
                        nc.sync.dma_start(
                            out=wl[:, b0 : b0 + nb, :],
                            in_=wh_d[
                                h, :, off * COLS : (off + nb) * COLS
                            ],
                        )
                        off += nb
                        if dma_only:
                            continue
                        last = ci == len(sched[h]) - 1
                        if no_tail and h == 1 and last:
                            continue
                        ah = a_hi - (sched[h][-1] // 2 if no_tail and h == 1 else 0)
                        for a in range(b0 // 2, (b0 + nb) // 2):
                            for s in range(NJ):
                                nc.tensor.matmul(
                                    pts[h][s],
                                    lhsT=ft_sb[:, 2 * a : 2 * a + 2, 0:M],
                                    rhs=wl[
                                        :,
                                        2 * a : 2 * a + 2,
                                        s * JT : (s + 1) * JT,
                                    ],
                                    start=(a == a_lo),
                                    stop=(a == ah - 1),
                                    perf_mode=DR,
                                )
                        if last and not no_dve and not no_tail:
                            dve_tail(h, pts)

            if runtime_reps:
                reps_sb = persist.tile([1, 2], mybir.dt.int32, name="reps_sb")
                nc.sync.dma_start(out=reps_sb, in_=reps_d)
                nreps = nc.values_load(
                    reps_sb[0:1, 0:1], min_val=0, max_val=1 << 20
                )
                rep_ctx = tc.For_i(0, nreps, 1)
            elif loop_reps > 1:
                rep_ctx = tc.For_i(0, loop_reps, 1)
            else:
                rep_ctx = contextlib.nullcontext()

            with rep_ctx:
                for _ in range(unroll):
                    if gram_in_loop:
                        gram_chain()
                    stream()
            nc.sync.dma_start(out=acc_d, in_=acc_sb)

    nc.compile()
    return nc


def _get_program():
    if "nc" not in _compiled:
        _compiled["nc"] = _build()
    return _compiled["nc"]


def _make_in_maps(w_batch, F_batch, B_batch):
    from concourse import mybir

    np_fp8 = mybir.dt.np(mybir.dt.float8e4)
    np_bf16 = mybir.dt.np(mybir.dt.bfloat16)

    w_batch = np.ascontiguousarray(w_batch, dtype=np.float32)
    F_batch = np.asarray(F_batch, dtype=np.float32)
    B_batch = np.asarray(B_batch, dtype=np.float32)

    fn = (F_batch.astype(np.float64) ** 2).sum(axis=0)  # [n] col sq-norms
    bn = (B_batch.astype(np.float64) ** 2).sum(axis=0)

    w8 = w_batch.astype(np_fp8)

    faug = np.zeros((MP, BATCH), dtype=np.float32)
    faug[0:K] = F_batch
    faug[K] = 1.0
    faug[K + 1] = (fn - 64.0).astype(np.float32)
    # ft[p, b, m] = Faug[m, b*128 + p]  (m padded to MP)
    ft = np.ascontiguousarray(
        faug.astype(np_fp8).T.reshape(NB, KP, MP).transpose(1, 0, 2)
    )

    baug = np.empty((M, BATCH), dtype=np.float32)
    baug[0:K] = B_batch
    baug[K] = (bn + 64.0).astype(np.float32)
    baug[K + 1] = 1.0
    baug16 = baug.astype(np_bf16)

    in_maps = []
    for c in range(NCORES):
        lo, hi = c * COLS, (c + 1) * COLS
        # wh[h, p, b*COLS + j] = w[(h*HB + b)*128 + p, lo + j]
        wh = np.ascontiguousarray(
            w8[:, lo:hi]
            .reshape(2, HB * KP, COLS)
            .reshape(2, HB, KP, COLS)
            .transpose(0, 2, 1, 3)
            .reshape(2, KP, HB * COLS)
        )
        in_maps.append(
            {
                "wh": wh,
                "ft": ft,
                "baug": np.ascontiguousarray(baug16[:, lo:hi]),
            }
        )
    return in_maps


def _combine(results):
    n = float(BATCH)
    S = np.zeros(M, dtype=np.float64)
    for r in results:
        S += r["acc"].astype(np.float64).sum(axis=1)

    cross = S[0:K].sum()
    # the +/- 64*sum(w) terms in rows 64/65 cancel exactly here
    tr_loss = S[K] + S[K + 1] - 2.0 * cross

    g0 = results[0]["gram"].astype(np.float64)  # identical on every core
    gram = g0[:, 0:K]
    rs = g0[:, K]
    g = gram / n - np.eye(K, dtype=np.float64)
    oth_loss = (g * g).sum()
    bla_loss = (rs * rs).sum()

    loss = (
        0.5 * tr_loss / (n * n) * 10000.0
        + 0.5 * bla_loss / n
        + 0.5 * oth_loss / K
    )
    return np.float32(loss)


def _ping_devices():
    """Touch every core with a trivial op first: a device wedged by a
    previously crashed process fails its next operation once and then
    recovers, so absorb that failure here instead of in the real run."""
    import time

    import jax

    for _ in range(3):
        try:
            for d in jax.devices()[:NCORES]:
                x = jax.device_put(np.ones(4, np.float32), d)
                (x + 1.0).block_until_ready()
            return
        except Exception:
            time.sleep(2.0)


def kernel(w_batch, F_batch, B_batch):
    import time

    from concourse.bass_utils import run_bass_kernel_spmd

    nc = _get_program()
    in_maps = _make_in_maps(w_batch, F_batch, B_batch)
    _ping_devices()
    try:
        res = run_bass_kernel_spmd(nc, in_maps, core_ids=list(range(NCORES)))
    except Exception:
        time.sleep(2.0)
        _ping_devices()
        res = run_bass_kernel_spmd(nc, in_maps, core_ids=list(range(NCORES)))
    return _combine(res.results)


# revision 54
# speedup vs baseline: 1.0142x; 1.0142x over previous
"""Trainium2 Bass kernel for the DAGH sample loss.

loss = 0.5 * tr_loss / n^2 * 1e4 + 0.5 * bla_loss / n + 0.5 * oth_loss / K

with
  tr_loss  = dot(rowsum(w), fn) + dot(colsum(w), bn) - 2 * sum((F @ w) * B)
  oth_loss = ||F F^T / n - I||_F^2
  bla_loss = sum_k (sum_i F[k, i])^2

Strategy (8 cores, SPMD): the problem is memory-bound on streaming w
(256 MB in f32), so w is quantized host-side to fp8-e4m3 (the 2e-2
rel-err gate leaves ~3 orders of magnitude of headroom; measured impact
is ~3e-5) and sharded COLUMN-wise: core c owns w[:, c*1024:(c+1)*1024],
pre-transposed on host so each SBUF partition receives contiguous 8 KB
DMA lines (32 KB partition stride measured fastest; the whole 8.4 MB
shard is SBUF-resident).

All three w-dependent reductions collapse into one augmented fp8
matmul chain per core, running in DoubleRow mode (two 128-row k-tiles
per instruction, 0.5 PE cycles/row):

  Faug = [F; ones; fn - 64]  (66 x 8192, fp8)  ->  out = Faug @ w_cols

  rows 0..63 . B          -> cross partial
  row  64    . (bn + 64)  -> colsum-dot partial + 64 * sum(w)
  row  65    . ones       -> rowsum-dot partial - 64 * sum(w)

(fn is centered by 64 so fp8 keeps ~6x more precision on it; the
64*sum(w) terms cancel exactly in tr_loss = S[64] + S[65] - 2*cross.)
The contraction is split into two PSUM groups so the first group's
multiply-by-Baug + reduce runs mid-stream, hidden under DMA; the final
chunks taper (8,8,8,4,2,2 blocks) so almost no matmul work trails the
last DMA byte.  Gram = F F^T and the F row-sums come from one extra
fp8 matmul chain over the same resident Faug^T tiles (every core
computes the identical full Gram; the host uses core 0's).  Host
combines the per-core scalar partials in f64.
"""

import numpy as np

BATCH = 8192
K = 64
NCORES = 8
COLS = BATCH // NCORES  # w columns per core (column sharding)
KP = 128  # partition dim
NB = BATCH // KP  # 64 row-blocks of w = full contraction per core
HB = NB // 2  # blocks per half-tensor (32 KB partition stride)
JT = 512  # psum free-dim tile
NJ = COLS // JT  # psum tiles per core (2)
M = K + 2  # augmented lhs rows (F, ones, fn-64)
MP = 80  # ft block stride: dual-fp8 ldweights needs k-subtile step % 16 == 0
CB = 8  # row-blocks per w DMA chunk (8 KB/partition lines)
NG = 2  # psum accumulation groups (= the two half-tensors)

_compiled = {}


def _chunk_sched(cb, taper):
    """Per-half chunk sizes in blocks.  half 1 optionally tapers so the
    final DMA chunks are small and almost no compute trails them."""
    uniform = [cb] * (HB // cb)
    if not taper:
        return [uniform, list(uniform)]
    tail = [cb // 2, cb // 4, cb // 4] if cb >= 4 else [cb]
    head = [cb] * ((HB - sum(tail)) // cb)
    assert sum(head) + sum(tail) == HB
    return [uniform, head + tail]


def _build(
    loop_reps=1,
    runtime_reps=False,
    dma_only=False,
    dma_tile=None,
    loop_whole=False,
    cb=CB,
    no_dve=False,
    no_tail=False,
    taper=True,
    dve_mode="mr16",
    unroll=1,
    gram_in_loop=False,
    jsplit=True,
):
    """loop_reps > 1 (or runtime_reps=True with a "reps" input tensor)
    wraps the main stream in a hardware For_i loop that recomputes
    identical results -- used only by test.py to time the steady-state
    stream without per-call NEFF-load overhead.  dma_only strips compute
    to measure the pure DMA bandwidth ceiling; no_dve / no_tail strip
    the tail stages for profiling."""
    import contextlib

    import concourse.bacc as bacc
    import concourse.mybir as mybir
    import concourse.tile as tile

    f32 = mybir.dt.float32
    bf16 = mybir.dt.bfloat16
    fp8 = mybir.dt.float8e4
    DR = mybir.MatmulPerfMode.DoubleRow

    sched = _chunk_sched(cb, taper)

    nc = bacc.Bacc(
        "TRN2", target_bir_lowering=False, debug=False, num_devices=NCORES
    )

    # w shard pre-transposed host-side, split into two half-tensors so the
    # DMA partition stride is 32 KB (measured faster than 64 KB):
    # wh[h, p, b*COLS + j] = w_cols[(h*HB + b)*128 + p, j]
    wh_d = nc.dram_tensor(
        "wh", [2, KP, HB * COLS], fp8, kind="ExternalInput"
    ).ap()
    # full Faug^T tiles: ft[p, b, m] = Faug[m, b*128 + p]  (m padded to MP)
    ft_d = nc.dram_tensor("ft", [KP, NB, MP], fp8, kind="ExternalInput").ap()
    # Baug column slice: [B; bn + 64; ones] for this core's columns
    baug_d = nc.dram_tensor(
        "baug", [M, COLS], bf16, kind="ExternalInput"
    ).ap()
    reps_d = None
    if runtime_reps:
        reps_d = nc.dram_tensor(
            "reps", [1, 2], mybir.dt.int32, kind="ExternalInput"
        ).ap()
    acc_d = nc.dram_tensor(
        "acc", [M, NG * NJ], f32, kind="ExternalOutput"
    ).ap()
    gram_d = nc.dram_tensor("gram", [K, M], f32, kind="ExternalOutput").ap()

    with tile.TileContext(nc) as tc:
        with (
            tc.tile_pool(name="persist", bufs=1) as persist,
            tc.tile_pool(name="wp", bufs=unroll) as wp,
            tc.tile_pool(name="scratch", bufs=2) as scratch,
            tc.tile_pool(name="psum", bufs=NG * NJ, space="PSUM") as psum,
            tc.tile_pool(name="psum_small", bufs=1, space="PSUM") as psum_small,
        ):
            ft_sb = persist.tile([KP, NB, MP], fp8, name="ft_sb")
            baug_sb = persist.tile([M, COLS], bf16, name="baug_sb")
            acc_sb = persist.tile([M, NG * NJ], f32, name="acc_sb")
            if dma_only or no_dve or no_tail:
                nc.vector.memset(acc_sb, 0.0)

            # preamble on the scalar-engine HWDGE ring: keeps these loads
            # off the sync ring so the w chunks aren't queued behind them
            nc.scalar.dma_start(out=ft_sb, in_=ft_d)
            nc.scalar.dma_start(out=baug_sb, in_=baug_d)

            # gram + F row-sums, fused: out[k, m] = sum_i F[k,i]*Faug[m,i]
            # (cols 0..63 = F F^T, col 64 = rowsum(F), col 65 ignored).
            # Tensor-engine work that overlaps the first w chunk DMAs.
            # Normal mode: DoubleRow loses below FD=128 (ldweights dominates).
            def gram_chain():
                gram_pt = psum_small.tile([K, M], f32, name="gram_pt")
                for b in range(NB):
                    nc.tensor.matmul(
                        gram_pt,
                        lhsT=ft_sb[:, b, 0:K],
                        rhs=ft_sb[:, b, 0:M],
                        start=(b == 0),
                        stop=(b == NB - 1),
                    )
                gram_sb = persist.tile([K, M], f32, name="gram_sb")
                nc.vector.tensor_copy(gram_sb, gram_pt)
                nc.scalar.dma_start(out=gram_d, in_=gram_sb)

            if not gram_in_loop:
                gram_chain()

            def dve_tail(g, pts, ss=None):
                # multiply group g's completed psum by Baug and reduce over
                # the free dim into acc.  (baseline found fused
                # tensor_tensor_reduce faults on HW with a PSUM input, so
                # the default "mr" keeps multiply and reduce separate)
                for s in range(NJ) if ss is None else ss:
                    bslice = baug_sb[:, s * JT : (s + 1) * JT]
                    acol = acc_sb[:, g * NJ + s : g * NJ + s + 1]
                    if dve_mode == "actmr":
                        # ACT engine drains PSUM -> SBUF as bf16 (2x DVE
                        # rate downstream); DVE multiplies and reduces.
                        # The two engines pipeline across the two s tiles.
                        st = scratch.tile([M, JT], bf16, name="act_out")
                        nc.scalar.copy(st, pts[g][s])
                        st2 = scratch.tile([M, JT], bf16, name="mul_out")
                        nc.vector.tensor_mul(st2, st, bslice)
                        nc.vector.tensor_reduce(
                            out=acol,
                            in_=st2,
                            axis=mybir.AxisListType.X,
                            op=mybir.AluOpType.add,
                        )
                    elif dve_mode == "mr16":
                        # like mr but the product scratch is bf16, halving
                        # the DVE reduce pass
                        st = scratch.tile([M, JT], bf16, name="mul_out")
                        nc.vector.tensor_mul(st, pts[g][s], bslice)
                        nc.vector.tensor_reduce(
                            out=acol,
                            in_=st,
                            axis=mybir.AxisListType.X,
                            op=mybir.AluOpType.add,
                        )
                    else:
                        # "mr": separate f32 multiply and reduce.  (The
                        # fused tensor_tensor_reduce instruction hangs this
                        # hardware even with SBUF-only inputs — do not use.)
                        st = scratch.tile([M, JT], f32, name="mul_out32")
                        nc.vector.tensor_mul(st, pts[g][s], bslice)
                        nc.vector.tensor_reduce(
                            out=acol,
                            in_=st,
                            axis=mybir.AxisListType.X,
                            op=mybir.AluOpType.add,
                        )

            def stream():
                wl = wp.tile([KP, NB, COLS], fp8, name="wl")
                pts = [
                    [psum.tile([M, JT], f32, name="mm_out") for _ in range(NJ)]
                    for _ in range(NG)
                ]
                for h in range(2):
                    off = 0  # block offset within this half
                    a_lo, a_hi = h * HB // 2, (h + 1) * HB // 2
                    for ci, nb in enumerate(sched[h]):
                        b0 = h * HB + off  # absolute block
                        last = ci == len(sched[h]) - 1
                        if jsplit and h == 1 and last and not dma_only:
                            # final chunk split by j-halves: s=0's psum
                            # closes (and its multiply+reduce runs) while
                            # s=1's half is still streaming
                            wh3 = wh_d[h].rearrange(
                                "p (b j) -> p b j", j=COLS
                            )
                            a = b0 // 2
                            for s in range(NJ):
                                nc.sync.dma_start(
                                    out=wl[
                                        :,
                                        b0 : b0 + nb,
                                        s * JT : (s + 1) * JT,
                                    ],
                                    in_=wh3[
                                        :,
                                        off : off + nb,
                                        s * JT : (s + 1) * JT,
                                    ],
                                )
                                nc.tensor.matmul(
                                    pts[h][s],
                                    lhsT=ft_sb[:, 2 * a : 2 * a + 2, 0:M],
                                    rhs=wl[
                                        :,
                                        2 * a : 2 * a + 2,
                                        s * JT : (s + 1) * JT,
                                    ],
                                    start=(a == a_lo),
                                    stop=True,
                                    perf_mode=DR,
                                )
                                dve_tail(h, pts, [s])
                            break
                        nc.sync.dma_start(
                            out=wl[:, b0 : b0 + nb, :],
                            in_=wh_d[
                                h, :, off * COLS : (off + nb) * COLS
                            ],
                        )
                        off += nb
                        if dma_only:
                            continue
                        last = ci == len(sched[h]) - 1
                        if no_tail and h == 1 and last:
                            continue
                        ah = a_hi - (sched[h][-1] // 2 if no_tail and h == 1 else 0)
                        for a in range(b0 // 2, (b0 + nb) // 2):
                            for s in range(NJ):
                                nc.tensor.matmul(
                                    pts[h][s],
                                    lhsT=ft_sb[:, 2 * a : 2 * a + 2, 0:M],
                                    rhs=wl[
                                        :,
                                        2 * a : 2 * a + 2,
                                        s * JT : (s + 1) * JT,
                                    ],
                                    start=(a == a_lo),
                                    stop=(a == ah - 1),
                                    perf_mode=DR,
                                )
                        if last and not no_dve and not no_tail:
                            dve_tail(h, pts)

            if runtime_reps:
                reps_sb = persist.tile([1, 2], mybir.dt.int32, name="reps_sb")
                nc.sync.dma_start(out=reps_sb, in_=reps_d)
                nreps = nc.values_load(
                    reps_sb[0:1, 0:1], min_val=0, max_val=1 << 20
                )
                rep_ctx = tc.For_i(0, nreps, 1)
            elif loop_reps > 1:
                rep_ctx = tc.For_i(0, loop_reps, 1)
            else:
                rep_ctx = contextlib.nullcontext()

            with rep_ctx:
                for _ in range(unroll):
                    if gram_in_loop:
                        gram_chain()
                    stream()
            nc.sync.dma_start(out=acc_d, in_=acc_sb)

    nc.compile()
    return nc


def _get_program():
    if "nc" not in _compiled:
        _compiled["nc"] = _build()
    return _compiled["nc"]


def _make_in_maps(w_batch, F_batch, B_batch):
    from concourse import mybir

    np_fp8 = mybir.dt.np(mybir.dt.float8e4)
    np_bf16 = mybir.dt.np(mybir.dt.bfloat16)

    w_batch = np.ascontiguousarray(w_batch, dtype=np.float32)
    F_batch = np.asarray(F_batch, dtype=np.float32)
    B_batch = np.asarray(B_batch, dtype=np.float32)

    fn = (F_batch.astype(np.float64) ** 2).sum(axis=0)  # [n] col sq-norms
    bn = (B_batch.astype(np.float64) ** 2).sum(axis=0)

    w8 = w_batch.astype(np_fp8)

    faug = np.zeros((MP, BATCH), dtype=np.float32)
    faug[0:K] = F_batch
    faug[K] = 1.0
    faug[K + 1] = (fn - 64.0).astype(np.float32)
    # ft[p, b, m] = Faug[m, b*128 + p]  (m padded to MP)
    ft = np.ascontiguousarray(
        faug.astype(np_fp8).T.reshape(NB, KP, MP).transpose(1, 0, 2)
    )

    baug = np.empty((M, BATCH), dtype=np.float32)
    baug[0:K] = B_batch
    baug[K] = (bn + 64.0).astype(np.float32)
    baug[K + 1] = 1.0
    baug16 = baug.astype(np_bf16)

    in_maps = []
    for c in range(NCORES):
        lo, hi = c * COLS, (c + 1) * COLS
        # wh[h, p, b*COLS + j] = w[(h*HB + b)*128 + p, lo + j]
        wh = np.ascontiguousarray(
            w8[:, lo:hi]
            .reshape(2, HB * KP, COLS)
            .reshape(2, HB, KP, COLS)
            .transpose(0, 2, 1, 3)
            .reshape(2, KP, HB * COLS)
        )
        in_maps.append(
            {
                "wh": wh,
                "ft": ft,
                "baug": np.ascontiguousarray(baug16[:, lo:hi]),
            }
        )
    return in_maps


def _combine(results):
    n = float(BATCH)
    S = np.zeros(M, dtype=np.float64)
    for r in results:
        S += r["acc"].astype(np.float64).sum(axis=1)

    cross = S[0:K].sum()
    # the +/- 64*sum(w) terms in rows 64/65 cancel exactly here
    tr_loss = S[K] + S[K + 1] - 2.0 * cross

    g0 = results[0]["gram"].astype(np.float64)  # identical on every core
    gram = g0[:, 0:K]
    rs = g0[:, K]
    g = gram / n - np.eye(K, dtype=np.float64)
    oth_loss = (g * g).sum()
    bla_loss = (rs * rs).sum()

    loss = (
        0.5 * tr_loss / (n * n) * 10000.0
        + 0.5 * bla_loss / n
        + 0.5 * oth_loss / K
    )
    return np.float32(loss)


def _ping_devices():
    """Touch every core with a trivial op first: a device wedged by a
    previously crashed process fails its next operation once and then
    recovers, so absorb that failure here instead of in the real run."""
    import time

    import jax

    for _ in range(3):
        try:
            for d in jax.devices()[:NCORES]:
                x = jax.device_put(np.ones(4, np.float32), d)
                (x + 1.0).block_until_ready()
            return
        except Exception:
            time.sleep(2.0)


def kernel(w_batch, F_batch, B_batch):
    import time

    from concourse.bass_utils import run_bass_kernel_spmd

    nc = _get_program()
    in_maps = _make_in_maps(w_batch, F_batch, B_batch)
    _ping_devices()
    try:
        res = run_bass_kernel_spmd(nc, in_maps, core_ids=list(range(NCORES)))
    except Exception:
        time.sleep(2.0)
        _ping_devices()
        res = run_bass_kernel_spmd(nc, in_maps, core_ids=list(range(NCORES)))
    return _combine(res.results)
